# revision 7
# baseline (speedup 1.0000x reference)
"""Multi-head attention (B=2, S=2048, D=768, H=12) on 8 trn2 NeuronCores.

Sharding: data-parallel over batch (2) x tensor-parallel over heads (4 groups
of 3 heads) = 8 cores. Each core projects Q/K/V for its head group from the
full activations, runs masked softmax attention, writes its slice of the attn
output, and computes a partial output projection (its heads' columns of Wo).
The host sums the 4 partial projections per batch element (the "all-reduce")
and adds the output bias.

Matmul operands are bf16 (fp32 matmuls run as two PE passes on trn2); PSUM
accumulation and the whole softmax/attn-output path stay fp32. Activations are
loaded pre-transposed via xbar DMA transpose (bf16-only HW path).

The mask is classified on the host: causal (tril) -> compile-time triangular
loop bounds, upper triangle of attn left to the runtime's zero-initialized
output buffers; all-ones -> full attention, no penalty; anything else -> an
additive -8e9 penalty tensor is shipped and added to the raw scores.
"""

import contextlib
import math

import ml_dtypes
import numpy as np

import concourse.bass as bass
import concourse.tile as tile
from concourse import bacc, mybir
from concourse.bass_utils import run_bass_kernel_spmd

B = 2
S = 2048
D = 768
H = 12
DK = 64
N_CORES = 8
HGROUPS = N_CORES // B          # 4 head groups
HG = H // HGROUPS               # 3 heads per core
DH = HG * DK                    # 192 projected features per core
SCALE = 1.0 / math.sqrt(DK)
PEN = -8.0e9                    # additive penalty; * SCALE = -1e9 like the ref

F32 = mybir.dt.float32
BF16 = mybir.dt.bfloat16
P = 128                         # partitions
NQT = S // P                    # 16 q tiles
NFC = D // P                    # 6 feature chunks
NSC = S // 512                  # 4 score chunks of 512

MODE_CAUSAL = 0
MODE_NONE = 1
MODE_GENERAL = 2

_cache: dict[int, object] = {}


def _build(mode: int):
    nc = bacc.Bacc("TRN2", target_bir_lowering=False, debug=False,
                   num_devices=N_CORES)

    xq = nc.dram_tensor("xq", [S, D], BF16, kind="ExternalInput")
    xk = nc.dram_tensor("xk", [S, D], BF16, kind="ExternalInput")
    xv = nc.dram_tensor("xv", [S, D], BF16, kind="ExternalInput")
    wqT = nc.dram_tensor("wqT", [D, DH], BF16, kind="ExternalInput")
    wkT = nc.dram_tensor("wkT", [D, DH], BF16, kind="ExternalInput")
    wvT = nc.dram_tensor("wvT", [D, DH], BF16, kind="ExternalInput")
    woT = nc.dram_tensor("woT", [DH, D], BF16, kind="ExternalInput")
    bqd = nc.dram_tensor("bqd", [DH, 1], F32, kind="ExternalInput")
    bkd = nc.dram_tensor("bkd", [DH, 1], F32, kind="ExternalInput")
    bvb = nc.dram_tensor("bvb", [P, DH], F32, kind="ExternalInput")
    pend = None
    if mode == MODE_GENERAL:
        pend = nc.dram_tensor("pen", [S, S], F32, kind="ExternalInput")

    attn_o = nc.dram_tensor("attn_o", [HG, S, S], F32, kind="ExternalOutput")
    out_o = nc.dram_tensor("out_o", [S, D], F32, kind="ExternalOutput")

    ident_d = nc.inline_tensor(
        np.eye(P, dtype=np.float32).astype(ml_dtypes.bfloat16), name="ident")
    # diag-block penalty: 0 where col<=row else PEN (strict upper triangle)
    pen_np = np.where(np.tril(np.ones((P, P), np.bool_)), 0.0, PEN)
    pen_d = nc.inline_tensor(pen_np.astype(np.float32), name="pen_diag")

    Exp = mybir.ActivationFunctionType.Exp
    X = mybir.AxisListType.X

    with tile.TileContext(nc) as tc:
        consts_cm = tc.tile_pool(name="consts", bufs=1)
        consts = consts_cm.__enter__()
        ident = consts.tile([P, P], BF16, tag="ident")
        nc.sync.dma_start(out=ident, in_=ident_d[:, :])
        pen_sb = consts.tile([P, P], F32, tag="pen")
        nc.sync.dma_start(out=pen_sb, in_=pen_d[:, :])
        bq_a = consts.tile([P, 1], F32, tag="bq_a")
        bq_b = consts.tile([DK, 1], F32, tag="bq_b")
        bk_a = consts.tile([P, 1], F32, tag="bk_a")
        bk_b = consts.tile([DK, 1], F32, tag="bk_b")
        nc.sync.dma_start(out=bq_a, in_=bqd[0:P, :])
        nc.sync.dma_start(out=bq_b, in_=bqd[P:DH, :])
        nc.sync.dma_start(out=bk_a, in_=bkd[0:P, :])
        nc.sync.dma_start(out=bk_b, in_=bkd[P:DH, :])
        ones_sb = consts.tile([1, DK], BF16, tag="ones")
        nc.vector.memset(ones_sb, 1.0)
        bv_sb = consts.tile([P, DH], F32, tag="bv")
        nc.sync.dma_start(out=bv_sb, in_=bvb[:, :])

        # weights, [D, DH] viewed as [NFC, 128, DH]
        wq_sb = consts.tile([P, NFC, DH], BF16, tag="wq")
        wk_sb = consts.tile([P, NFC, DH], BF16, tag="wk")
        wv_sb = consts.tile([P, NFC, DH], BF16, tag="wv")
        for (wd, wt) in ((wqT, wq_sb), (wkT, wk_sb), (wvT, wv_sb)):
            wr = wd.rearrange("(c p) m -> c p m", p=P)
            for c in range(NFC):
                nc.sync.dma_start(out=wt[:, c, :], in_=wr[c])
        # woT [DH, D] -> [128, D] + [64, D]
        wo_a = consts.tile([P, D], BF16, tag="wo_a")
        wo_b = consts.tile([DK, D], BF16, tag="wo_b")
        nc.sync.dma_start(out=wo_a, in_=woT[0:P, :])
        nc.sync.dma_start(out=wo_b, in_=woT[P:DH, :])

        # persistent activations
        persist_cm = tc.tile_pool(name="persist", bufs=1)
        persist = persist_cm.__enter__()
        qt_a = persist.tile([P, S], BF16, tag="qt_a")   # heads 0,1 (dk rows)
        qt_b = persist.tile([DK, S], BF16, tag="qt_b")  # head 2
        kt_a = persist.tile([P, S], BF16, tag="kt_a")
        kt_b = persist.tile([DK, S], BF16, tag="kt_b")
        v_sb = persist.tile([P, NQT, DH], BF16, tag="v")  # [s%128, s//128, dh]
        cx_a = persist.tile([P, S], BF16, tag="cx_a")   # ctx^T heads 0,1
        cx_b = persist.tile([DK, S], BF16, tag="cx_b")  # ctx^T head 2

        # ---------------- phase A: projections -----------------------------
        with (
            tc.tile_pool(name="xt", bufs=1) as xtp,
            tc.tile_pool(name="ps_mm", bufs=2, space="PSUM") as ps_mmp,
            tc.tile_pool(name="ps_sm", bufs=2, space="PSUM") as ps_smp,
        ):
            for which, xd in (("k", xk), ("q", xq), ("v", xv)):
                # xbar DMA transpose: x[s, f-chunk] -> xT chunk [128f, S]
                xt_sb = xtp.tile([P, NFC, S], BF16, tag="xt")
                for c in range(NFC):
                    nc.sync.dma_start(out=xt_sb[:, c, :],
                                      in_=xd[:, c * P:(c + 1) * P],
                                      transpose=True)
                if which in ("q", "k"):
                    w_sb = wq_sb if which == "q" else wk_sb
                    b_a = bq_a if which == "q" else bk_a
                    b_b = bq_b if which == "q" else bk_b
                    o_a = qt_a if which == "q" else kt_a
                    o_b = qt_b if which == "q" else kt_b
                    for sc in range(NSC):
                        ssl = slice(sc * 512, (sc + 1) * 512)
                        pa = ps_mmp.tile([P, 512], F32, tag="pm")
                        for c in range(NFC):
                            nc.tensor.matmul(pa, w_sb[:, c, 0:P],
                                             xt_sb[:, c, ssl],
                                             start=(c == 0), stop=(c == NFC - 1))
                        nc.scalar.add(o_a[:, ssl], pa, b_a)
                        pb = ps_smp.tile([P, 512], F32, tag="pb")
                        for c in range(NFC):
                            nc.tensor.matmul(pb[0:DK, :], w_sb[:, c, P:DH],
                                             xt_sb[:, c, ssl],
                                             start=(c == 0), stop=(c == NFC - 1))
                        nc.scalar.add(o_b[:, ssl], pb[0:DK, :], b_b)
                else:
                    for st in range(NQT):
                        pv = ps_smp.tile([P, 512], F32, tag="pb")
                        for c in range(NFC):
                            nc.tensor.matmul(
                                pv[:, 0:DH],
                                xt_sb[:, c, st * P:(st + 1) * P],
                                wv_sb[:, c, :],
                                start=(c == 0), stop=(c == NFC - 1))
                        nc.vector.tensor_add(v_sb[:, st, :], pv[:, 0:DH], bv_sb)

        # ---------------- phase B: attention + output projection ----------
        XBAR_T = True                   # attn^T via DMA xbar (else PE matmul)
        CHW = 1024 if XBAR_T else 512   # exp/psum chunk width
        NQB = NQT // 4                  # q blocks of 4 q tiles
        with (
            tc.tile_pool(name="exp", bufs=5) as expp,
            tc.tile_pool(name="nrm", bufs=3) as nrmp,
            tc.tile_pool(name="at", bufs=6) as atp,
            tc.tile_pool(name="small", bufs=6) as smallp,
            tc.tile_pool(name="outp", bufs=2) as outp,
            tc.tile_pool(name="penp", bufs=2) as penp,
            tc.tile_pool(name="ps_s", bufs=2, space="PSUM") as ps_sp,
            tc.tile_pool(name="ps_c", bufs=2, space="PSUM") as ps_cp,
            tc.tile_pool(name="ps_o", bufs=2, space="PSUM") as ps_op,
            tc.tile_pool(name="ps_t2", bufs=2, space="PSUM") if not XBAR_T
            else contextlib.nullcontext() as ps_tp2,
        ):
            def q_head(h, sl):
                if h < 2:
                    return qt_a[h * DK:(h + 1) * DK, sl]
                return qt_b[:, sl]

            def k_head(h, sl):
                if h < 2:
                    return kt_a[h * DK:(h + 1) * DK, sl]
                return kt_b[:, sl]

            for qb in range(NQB):
                exps = {}
                recs = {}
                for qi in range(4):
                    qt = qb * 4 + qi
                    q_sl = slice(qt * P, (qt + 1) * P)
                    ncol = (qt + 1) * P if mode == MODE_CAUSAL else S
                    nch = (ncol + CHW - 1) // CHW

                    pg_t = None
                    if mode == MODE_GENERAL:
                        pg_t = penp.tile([P, S], F32, tag="pg")
                        nc.sync.dma_start(out=pg_t, in_=pend[q_sl, :])

                    exp_h = [expp.tile([P, S], BF16, tag=f"exp{h}",
                                       name=f"exp{h}") for h in range(HG)]
                    acc_h = [smallp.tile([P, 2], F32, tag=f"acc{h}",
                                        name=f"acc{h}") for h in range(HG)]
                    for h in range(HG):
                        exps[(h, qi)] = exp_h[h]
                    for ch in range(nch):
                        ccols = min(CHW, ncol - ch * CHW)
                        nsub = (ccols + 511) // 512
                        ps_h = [ps_sp.tile([P, CHW], F32, tag="ps",
                                          name=f"ps{h2_}")
                                for h2_ in range(HG)]
                        # h0/h1 adjacent -> concurrent PE row groups
                        for sub in range(nsub):
                            cols = min(512, ccols - sub * 512)
                            c0 = ch * CHW + sub * 512
                            for h in range(HG):
                                nc.tensor.matmul(
                                    ps_h[h][:, sub * 512:sub * 512 + cols],
                                    q_head(h, q_sl), k_head(h, slice(c0, c0 + cols)),
                                    start=True, stop=True)
                        for h in range(HG):
                            if mode == MODE_CAUSAL and (qt * P) // CHW == ch:
                                off = qt * P - ch * CHW
                                nc.vector.tensor_add(ps_h[h][:, off:off + P],
                                                     ps_h[h][:, off:off + P],
                                                     pen_sb)
                            elif mode == MODE_GENERAL:
                                nc.vector.tensor_add(
                                    ps_h[h][:, 0:ccols], ps_h[h][:, 0:ccols],
                                    pg_t[:, ch * CHW:ch * CHW + ccols])
                            nc.scalar.activation(
                                exp_h[h][:, ch * CHW:ch * CHW + ccols],
                                ps_h[h][:, 0:ccols], Exp, scale=SCALE,
                                accum_out=acc_h[h][:, ch:ch + 1])

                    for h in range(HG):
                        red_t = smallp.tile([P, 1], F32, tag="red")
                        if nch > 1:
                            nc.vector.reduce_sum(red_t, acc_h[h][:, 0:nch],
                                                 axis=X)
                        else:
                            nc.vector.tensor_copy(red_t, acc_h[h][:, 0:1])
                        rec_t = smallp.tile([P, 1], F32, tag="rec")
                        nc.vector.reciprocal(rec_t, red_t)
                        rec_bf = smallp.tile([P, 1], BF16, tag=f"rb{h}")
                        nc.vector.tensor_copy(rec_bf, rec_t)
                        recs[(h, qi)] = rec_bf

                        nrm_t = nrmp.tile([P, S], F32, tag="nrm")
                        nc.vector.tensor_scalar_mul(nrm_t[:, 0:ncol],
                                                    exp_h[h][:, 0:ncol], rec_t)
                        nc.sync.dma_start(out=attn_o[h, q_sl, 0:ncol],
                                          in_=nrm_t[:, 0:ncol])

                # attn^T + ctx + recip scaling + cx, per head
                nkcb = qb * 4 + 4 if mode == MODE_CAUSAL else NQT
                qb_sl = slice(qb * 512, (qb + 1) * 512)
                for h in range(HG):
                    pc4 = ps_cp.tile([DK, 512], F32, tag="pc")
                    for kc in range(nkcb):
                        qi0 = max(kc - qb * 4, 0) if mode == MODE_CAUSAL else 0
                        at4 = atp.tile([P, 512], BF16, tag="at4")
                        if XBAR_T:
                            for qi in range(qi0, 4):
                                nc.sync.dma_start(
                                    out=at4[:, qi * P:(qi + 1) * P],
                                    in_=exps[(h, qi)][:, kc * P:(kc + 1) * P],
                                    transpose=True)
                        else:
                            pt2 = ps_tp2.tile([P, 512], F32, tag="pt2")
                            for qi in range(qi0, 4):
                                nc.tensor.matmul(
                                    pt2[:, qi * P:(qi + 1) * P],
                                    exps[(h, qi)][:, kc * P:(kc + 1) * P],
                                    ident, start=True, stop=True,
                                    is_transpose=True)
                            if kc % 2 == 0:
                                nc.scalar.copy(at4[:, qi0 * P:512],
                                               pt2[:, qi0 * P:512])
                            else:
                                nc.vector.tensor_copy(at4[:, qi0 * P:512],
                                                      pt2[:, qi0 * P:512])
                        nc.tensor.matmul(
                            pc4[:, qi0 * P:512],
                            v_sb[:, kc, h * DK:(h + 1) * DK],
                            at4[:, qi0 * P:512],
                            start=(kc == 0), stop=(kc == nkcb - 1),
                            skip_group_check=True)

                    # rb4 = ones(64)^T x recip-row(512), applied to pc4
                    recT_ps = ps_op.tile([1, 512], F32, tag="po")
                    for qi in range(4):
                        nc.tensor.matmul(recT_ps[:, qi * P:(qi + 1) * P],
                                         recs[(h, qi)], ident,
                                         start=True, stop=True)
                    recT_sb = smallp.tile([1, 512], BF16, tag="rt")
                    nc.scalar.copy(recT_sb, recT_ps)
                    rb_ps = ps_op.tile([DK, 512], F32, tag="po")
                    nc.tensor.matmul(rb_ps, ones_sb, recT_sb,
                                     start=True, stop=True)
                    rb_sb = smallp.tile([DK, 512], F32, tag="rbs")
                    nc.scalar.copy(rb_sb, rb_ps)
                    if h < 2:
                        nc.vector.tensor_mul(cx_a[h * DK:(h + 1) * DK, qb_sl],
                                             pc4, rb_sb)
                    else:
                        nc.vector.tensor_mul(cx_b[:, qb_sl], pc4, rb_sb)

                # output projection for the 4 q tiles of this block
                for qi in range(4):
                    qt = qb * 4 + qi
                    q_sl = slice(qt * P, (qt + 1) * P)
                    out_t = outp.tile([P, D], F32, tag="out")
                    for fo in range(2):
                        cols = 512 if fo == 0 else D - 512
                        fsl = slice(fo * 512, fo * 512 + cols)
                        po = ps_op.tile([P, 512], F32, tag="po")
                        nc.tensor.matmul(po[:, 0:cols], cx_a[:, q_sl],
                                         wo_a[:, fsl], start=True, stop=False)
                        nc.tensor.matmul(po[:, 0:cols], cx_b[:, q_sl],
                                         wo_b[:, fsl], start=False, stop=True)
                        nc.vector.tensor_copy(out_t[:, fsl], po[:, 0:cols])
                    nc.sync.dma_start(out=out_o[q_sl, :], in_=out_t)

        persist_cm.__exit__(None, None, None)
        consts_cm.__exit__(None, None, None)

    nc.compile()
    return nc


def _classify_mask(mask: np.ndarray) -> int:
    m = np.asarray(mask)
    if (m != 0).all():
        return MODE_NONE
    tril = np.tril(np.ones((S, S), np.bool_))
    if ((m != 0) == tril).all():
        return MODE_CAUSAL
    return MODE_GENERAL


def _bf(a):
    return np.ascontiguousarray(np.asarray(a, np.float32).astype(
        ml_dtypes.bfloat16))


def _run(inputs: dict, trace: bool = False):
    query = np.asarray(inputs["query"], np.float32)
    key = np.asarray(inputs["key"], np.float32)
    value = np.asarray(inputs["value"], np.float32)
    mask = np.asarray(inputs["mask"])
    wq = np.asarray(inputs["wq"], dtype=np.float32)
    wk = np.asarray(inputs["wk"], dtype=np.float32)
    wv = np.asarray(inputs["wv"], dtype=np.float32)
    wo = np.asarray(inputs["wo"], dtype=np.float32)
    bq = np.asarray(inputs["bq"], dtype=np.float32)
    bk = np.asarray(inputs["bk"], dtype=np.float32)
    bv = np.asarray(inputs["bv"], dtype=np.float32)
    bo = np.asarray(inputs["bo"], dtype=np.float32)

    mode = _classify_mask(mask)
    if mode not in _cache:
        _cache[mode] = _build(mode)
    nc = _cache[mode]

    pen_full = None
    if mode == MODE_GENERAL:
        pen_full = np.where(np.asarray(mask) == 0, np.float32(PEN),
                            np.float32(0.0))

    in_maps = []
    for core in range(N_CORES):
        b = core // HGROUPS
        hg = core % HGROUPS
        r0 = hg * DH
        rs = slice(r0, r0 + DH)
        m = {
            "xq": _bf(query[b]),
            "xk": _bf(key[b]),
            "xv": _bf(value[b]),
            "wqT": _bf(wq[rs, :].T),
            "wkT": _bf(wk[rs, :].T),
            "wvT": _bf(wv[rs, :].T),
            "woT": _bf(wo[:, rs].T),
            "bqd": np.ascontiguousarray(bq[rs].reshape(DH, 1)),
            "bkd": np.ascontiguousarray(bk[rs].reshape(DH, 1)),
            "bvb": np.ascontiguousarray(
                np.broadcast_to(bv[rs][None, :], (P, DH))),
        }
        if mode == MODE_GENERAL:
            m["pen"] = pen_full
        in_maps.append(m)

    res = run_bass_kernel_spmd(nc, in_maps, core_ids=list(range(N_CORES)),
                               trace=trace)

    attn = np.empty((B, H, S, S), np.float32)
    out = np.zeros((B, S, D), np.float32)
    for core in range(N_CORES):
        b = core // HGROUPS
        hg = core % HGROUPS
        r = res.results[core]
        attn[b, hg * HG:(hg + 1) * HG] = r["attn_o"]
        out[b] += r["out_o"]
    out += bo[None, None, :]
    return (out, attn), res


def kernel(**inputs):
    (out, attn), _ = _run(inputs)
    return out, attn


# revision 11
# speedup vs baseline: 2.4647x; 2.4647x over previous
"""Multi-head attention (B=2, S=2048, D=768, H=12) on 8 trn2 NeuronCores.

Sharding: data-parallel over batch (2) x tensor-parallel over heads (4 groups
of 3 heads) = 8 cores. Each core projects Q/K/V for its head group from the
full activations, runs masked softmax attention, writes its slice of the attn
output, and computes a partial output projection (its heads' columns of Wo).
The host sums the 4 partial projections per batch element (the "all-reduce")
and adds the output bias.

Matmul operands are bf16 (fp32 matmuls run as two PE passes on trn2); PSUM
accumulation and the whole softmax/attn-output path stay fp32. Activations are
loaded pre-transposed via xbar DMA transpose (bf16-only HW path).

The mask is classified on the host: causal (tril) -> compile-time triangular
loop bounds, upper triangle of attn left to the runtime's zero-initialized
output buffers; all-ones -> full attention, no penalty; anything else -> an
additive -8e9 penalty tensor is shipped and added to the raw scores.
"""

import contextlib
import math

import ml_dtypes
import numpy as np

import concourse.bass as bass
import concourse.tile as tile
from concourse import bacc, mybir
from concourse.bass_utils import run_bass_kernel_spmd

B = 2
S = 2048
D = 768
H = 12
DK = 64
N_CORES = 8
HGROUPS = N_CORES // B          # 4 head groups
HG = H // HGROUPS               # 3 heads per core
DH = HG * DK                    # 192 projected features per core
SCALE = 1.0 / math.sqrt(DK)
PEN = -8.0e9                    # additive penalty; * SCALE = -1e9 like the ref

F32 = mybir.dt.float32
BF16 = mybir.dt.bfloat16
P = 128                         # partitions
NQT = S // P                    # 16 q tiles
NFC = D // P                    # 6 feature chunks
NSC = S // 512                  # 4 score chunks of 512

MODE_CAUSAL = 0
MODE_NONE = 1
MODE_GENERAL = 2

_cache: dict[int, object] = {}


def _build(mode: int):
    nc = bacc.Bacc("TRN2", target_bir_lowering=False, debug=False,
                   num_devices=N_CORES)

    xq = nc.dram_tensor("xq", [S, D], BF16, kind="ExternalInput")
    xk = nc.dram_tensor("xk", [S, D], BF16, kind="ExternalInput")
    xv = nc.dram_tensor("xv", [S, D], BF16, kind="ExternalInput")
    wqT = nc.dram_tensor("wqT", [D, DH], BF16, kind="ExternalInput")
    wkT = nc.dram_tensor("wkT", [D, DH], BF16, kind="ExternalInput")
    wvT = nc.dram_tensor("wvT", [D, DH], BF16, kind="ExternalInput")
    woT = nc.dram_tensor("woT", [DH, D], BF16, kind="ExternalInput")
    bqd = nc.dram_tensor("bqd", [DH, 1], F32, kind="ExternalInput")
    bkd = nc.dram_tensor("bkd", [DH, 1], F32, kind="ExternalInput")
    bvb = nc.dram_tensor("bvb", [P, DH], F32, kind="ExternalInput")
    pend = None
    if mode == MODE_GENERAL:
        pend = nc.dram_tensor("pen", [S, S], F32, kind="ExternalInput")

    attn_o = nc.dram_tensor("attn_o", [HG, S, S], F32, kind="ExternalOutput")
    out_o = nc.dram_tensor("out_o", [S, D], F32, kind="ExternalOutput")

    ident_d = nc.inline_tensor(
        np.eye(P, dtype=np.float32).astype(ml_dtypes.bfloat16), name="ident")
    # diag-block penalty: 0 where col<=row else PEN (strict upper triangle)
    pen_np = np.where(np.tril(np.ones((P, P), np.bool_)), 0.0, PEN)
    pen_d = nc.inline_tensor(pen_np.astype(np.float32), name="pen_diag")

    Exp = mybir.ActivationFunctionType.Exp
    X = mybir.AxisListType.X

    with tile.TileContext(nc) as tc:
        consts_cm = tc.tile_pool(name="consts", bufs=1)
        consts = consts_cm.__enter__()
        ident = consts.tile([P, P], BF16, tag="ident")
        nc.sync.dma_start(out=ident, in_=ident_d[:, :])
        pen_sb = consts.tile([P, P], F32, tag="pen")
        nc.sync.dma_start(out=pen_sb, in_=pen_d[:, :])
        bq_a = consts.tile([P, 1], F32, tag="bq_a")
        bq_b = consts.tile([DK, 1], F32, tag="bq_b")
        bk_a = consts.tile([P, 1], F32, tag="bk_a")
        bk_b = consts.tile([DK, 1], F32, tag="bk_b")
        nc.sync.dma_start(out=bq_a, in_=bqd[0:P, :])
        nc.sync.dma_start(out=bq_b, in_=bqd[P:DH, :])
        nc.sync.dma_start(out=bk_a, in_=bkd[0:P, :])
        nc.sync.dma_start(out=bk_b, in_=bkd[P:DH, :])
        ones_sb = consts.tile([1, DK], BF16, tag="ones")
        nc.vector.memset(ones_sb, 1.0)
        bv_sb = consts.tile([P, DH], F32, tag="bv")
        nc.sync.dma_start(out=bv_sb, in_=bvb[:, :])

        # weights, [D, DH] viewed as [NFC, 128, DH]
        wq_sb = consts.tile([P, NFC, DH], BF16, tag="wq")
        wk_sb = consts.tile([P, NFC, DH], BF16, tag="wk")
        wv_sb = consts.tile([P, NFC, DH], BF16, tag="wv")
        for (wd, wt) in ((wqT, wq_sb), (wkT, wk_sb), (wvT, wv_sb)):
            wr = wd.rearrange("(c p) m -> c p m", p=P)
            for c in range(NFC):
                nc.sync.dma_start(out=wt[:, c, :], in_=wr[c])
        # woT [DH, D] -> [128, D] + [64, D]
        wo_a = consts.tile([P, D], BF16, tag="wo_a")
        wo_b = consts.tile([DK, D], BF16, tag="wo_b")
        nc.sync.dma_start(out=wo_a, in_=woT[0:P, :])
        nc.sync.dma_start(out=wo_b, in_=woT[P:DH, :])

        # persistent activations
        persist_cm = tc.tile_pool(name="persist", bufs=1)
        persist = persist_cm.__enter__()
        qt_a = persist.tile([P, S], BF16, tag="qt_a")   # heads 0,1 (dk rows)
        qt_b = persist.tile([DK, S], BF16, tag="qt_b")  # head 2
        kt_a = persist.tile([P, S], BF16, tag="kt_a")
        kt_b = persist.tile([DK, S], BF16, tag="kt_b")
        v_sb = persist.tile([P, NQT, DH], BF16, tag="v")  # [s%128, s//128, dh]
        cx_a = persist.tile([P, S], BF16, tag="cx_a")   # ctx^T heads 0,1
        cx_b = persist.tile([DK, S], BF16, tag="cx_b")  # ctx^T head 2

        # ---------------- phase A: projections -----------------------------
        with (
            tc.tile_pool(name="xt", bufs=1) as xtp,
            tc.tile_pool(name="ps_mm", bufs=2, space="PSUM") as ps_mmp,
            tc.tile_pool(name="ps_sm", bufs=2, space="PSUM") as ps_smp,
        ):
            for which, xd in (("k", xk), ("q", xq), ("v", xv)):
                # xbar DMA transpose: x[s, f-chunk] -> xT chunk [128f, S]
                xt_sb = xtp.tile([P, NFC, S], BF16, tag="xt")
                for c in range(NFC):
                    nc.sync.dma_start(out=xt_sb[:, c, :],
                                      in_=xd[:, c * P:(c + 1) * P],
                                      transpose=True)
                if which in ("q", "k"):
                    w_sb = wq_sb if which == "q" else wk_sb
                    b_a = bq_a if which == "q" else bk_a
                    b_b = bq_b if which == "q" else bk_b
                    o_a = qt_a if which == "q" else kt_a
                    o_b = qt_b if which == "q" else kt_b
                    for sc in range(NSC):
                        ssl = slice(sc * 512, (sc + 1) * 512)
                        pa = ps_mmp.tile([P, 512], F32, tag="pm")
                        for c in range(NFC):
                            nc.tensor.matmul(pa, w_sb[:, c, 0:P],
                                             xt_sb[:, c, ssl],
                                             start=(c == 0), stop=(c == NFC - 1))
                        nc.scalar.add(o_a[:, ssl], pa, b_a)
                        pb = ps_smp.tile([P, 512], F32, tag="pb")
                        for c in range(NFC):
                            nc.tensor.matmul(pb[0:DK, :], w_sb[:, c, P:DH],
                                             xt_sb[:, c, ssl],
                                             start=(c == 0), stop=(c == NFC - 1))
                        nc.scalar.add(o_b[:, ssl], pb[0:DK, :], b_b)
                else:
                    for st in range(NQT):
                        pv = ps_smp.tile([P, 512], F32, tag="pb")
                        for c in range(NFC):
                            nc.tensor.matmul(
                                pv[:, 0:DH],
                                xt_sb[:, c, st * P:(st + 1) * P],
                                wv_sb[:, c, :],
                                start=(c == 0), stop=(c == NFC - 1))
                        nc.vector.tensor_add(v_sb[:, st, :], pv[:, 0:DH], bv_sb)

        # ---------------- phase B: attention + output projection ----------
        XBAR_T = False                  # attn^T via DMA xbar (else PE matmul)
        CHW = 1024                      # exp/psum chunk width
        NQB = NQT // 4                  # q blocks of 4 q tiles
        with (
            tc.tile_pool(name="exp", bufs=5) as expp,
            tc.tile_pool(name="nrm", bufs=3) as nrmp,
            tc.tile_pool(name="at", bufs=6) as atp,
            tc.tile_pool(name="small", bufs=6) as smallp,
            tc.tile_pool(name="outp", bufs=2) as outp,
            tc.tile_pool(name="penp", bufs=2) as penp,
            tc.tile_pool(name="ps_s", bufs=2, space="PSUM") as ps_sp,
            tc.tile_pool(name="ps_c", bufs=1, space="PSUM") as ps_cp,
            tc.tile_pool(name="ps_o", bufs=1, space="PSUM") as ps_op,
            tc.tile_pool(name="ps_t2", bufs=2, space="PSUM") if not XBAR_T
            else contextlib.nullcontext() as ps_tp2,
        ):
            def q_head(h, sl):
                if h < 2:
                    return qt_a[h * DK:(h + 1) * DK, sl]
                return qt_b[:, sl]

            def k_head(h, sl):
                if h < 2:
                    return kt_a[h * DK:(h + 1) * DK, sl]
                return kt_b[:, sl]

            for qb in range(NQB):
                exps = {}
                recs = {}
                for qi in range(4):
                    qt = qb * 4 + qi
                    q_sl = slice(qt * P, (qt + 1) * P)
                    ncol = (qt + 1) * P if mode == MODE_CAUSAL else S
                    nch = (ncol + CHW - 1) // CHW

                    pg_t = None
                    if mode == MODE_GENERAL:
                        pg_t = penp.tile([P, S], F32, tag="pg")
                        nc.sync.dma_start(out=pg_t, in_=pend[q_sl, :])

                    exp_h = [expp.tile([P, S], BF16, tag=f"exp{h}",
                                       name=f"exp{h}") for h in range(HG)]
                    acc_h = [smallp.tile([P, 2], F32, tag=f"acc{h}",
                                        name=f"acc{h}") for h in range(HG)]
                    for h in range(HG):
                        exps[(h, qi)] = exp_h[h]
                    for ch in range(nch):
                        ccols = min(CHW, ncol - ch * CHW)
                        nsub = (ccols + 511) // 512
                        ps_h = [ps_sp.tile([P, CHW], F32, tag="ps",
                                          name=f"ps{h2_}")
                                for h2_ in range(HG)]
                        # h0/h1 adjacent -> concurrent PE row groups
                        for sub in range(nsub):
                            cols = min(512, ccols - sub * 512)
                            c0 = ch * CHW + sub * 512
                            for h in range(HG):
                                nc.tensor.matmul(
                                    ps_h[h][:, sub * 512:sub * 512 + cols],
                                    q_head(h, q_sl), k_head(h, slice(c0, c0 + cols)),
                                    start=True, stop=True)
                        for h in range(HG):
                            if mode == MODE_CAUSAL and (qt * P) // CHW == ch:
                                off = qt * P - ch * CHW
                                nc.vector.tensor_add(ps_h[h][:, off:off + P],
                                                     ps_h[h][:, off:off + P],
                                                     pen_sb)
                            elif mode == MODE_GENERAL:
                                nc.vector.tensor_add(
                                    ps_h[h][:, 0:ccols], ps_h[h][:, 0:ccols],
                                    pg_t[:, ch * CHW:ch * CHW + ccols])
                            nc.scalar.activation(
                                exp_h[h][:, ch * CHW:ch * CHW + ccols],
                                ps_h[h][:, 0:ccols], Exp, scale=SCALE,
                                accum_out=acc_h[h][:, ch:ch + 1])

                    for h in range(HG):
                        red_t = smallp.tile([P, 1], F32, tag="red")
                        if nch > 1:
                            nc.vector.reduce_sum(red_t, acc_h[h][:, 0:nch],
                                                 axis=X)
                        else:
                            nc.vector.tensor_copy(red_t, acc_h[h][:, 0:1])
                        rec_t = smallp.tile([P, 1], F32, tag="rec")
                        nc.vector.reciprocal(rec_t, red_t)
                        rec_bf = smallp.tile([P, 1], BF16, tag=f"rb{h}")
                        nc.vector.tensor_copy(rec_bf, rec_t)
                        recs[(h, qi)] = rec_bf

                        nrm_t = nrmp.tile([P, S], F32, tag="nrm")
                        nc.vector.tensor_scalar_mul(nrm_t[:, 0:ncol],
                                                    exp_h[h][:, 0:ncol], rec_t)
                        nc.sync.dma_start(out=attn_o[h, q_sl, 0:ncol],
                                          in_=nrm_t[:, 0:ncol])

                # attn^T + ctx + recip scaling + cx, per head
                nkcb = qb * 4 + 4 if mode == MODE_CAUSAL else NQT
                qb_sl = slice(qb * 512, (qb + 1) * 512)
                for h in range(HG):
                    pc4 = ps_cp.tile([DK, 512], F32, tag="pc")
                    for kc in range(nkcb):
                        qi0 = max(kc - qb * 4, 0) if mode == MODE_CAUSAL else 0
                        at4 = atp.tile([P, 512], BF16, tag="at4")
                        if XBAR_T:
                            for qi in range(qi0, 4):
                                nc.sync.dma_start(
                                    out=at4[:, qi * P:(qi + 1) * P],
                                    in_=exps[(h, qi)][:, kc * P:(kc + 1) * P],
                                    transpose=True)
                        else:
                            pt2 = ps_tp2.tile([P, 512], F32, tag="pt2")
                            for qi in range(qi0, 4):
                                nc.tensor.matmul(
                                    pt2[:, qi * P:(qi + 1) * P],
                                    exps[(h, qi)][:, kc * P:(kc + 1) * P],
                                    ident, start=True, stop=True)
                            if kc % 2 == 0:
                                nc.scalar.copy(at4[:, qi0 * P:512],
                                               pt2[:, qi0 * P:512])
                            else:
                                nc.vector.tensor_copy(at4[:, qi0 * P:512],
                                                      pt2[:, qi0 * P:512])
                        nc.tensor.matmul(
                            pc4[:, qi0 * P:512],
                            v_sb[:, kc, h * DK:(h + 1) * DK],
                            at4[:, qi0 * P:512],
                            start=(kc == 0), stop=(kc == nkcb - 1),
                            skip_group_check=True)

                    # rb4 = ones(64)^T x recip-row(512), applied to pc4
                    recT_ps = ps_op.tile([1, 512], F32, tag="po")
                    for qi in range(4):
                        nc.tensor.matmul(recT_ps[:, qi * P:(qi + 1) * P],
                                         recs[(h, qi)], ident,
                                         start=True, stop=True)
                    recT_sb = smallp.tile([1, 512], BF16, tag="rt")
                    nc.scalar.copy(recT_sb, recT_ps)
                    rb_ps = ps_op.tile([DK, 512], F32, tag="po")
                    nc.tensor.matmul(rb_ps, ones_sb, recT_sb,
                                     start=True, stop=True)
                    rb_sb = smallp.tile([DK, 512], F32, tag="rbs")
                    nc.scalar.copy(rb_sb, rb_ps)
                    if h < 2:
                        nc.vector.tensor_mul(cx_a[h * DK:(h + 1) * DK, qb_sl],
                                             pc4, rb_sb)
                    else:
                        nc.vector.tensor_mul(cx_b[:, qb_sl], pc4, rb_sb)

                # output projection for the 4 q tiles of this block
                for qi in range(4):
                    qt = qb * 4 + qi
                    q_sl = slice(qt * P, (qt + 1) * P)
                    out_t = outp.tile([P, D], F32, tag="out")
                    for fo in range(2):
                        cols = 512 if fo == 0 else D - 512
                        fsl = slice(fo * 512, fo * 512 + cols)
                        po = ps_op.tile([P, 512], F32, tag="po")
                        nc.tensor.matmul(po[:, 0:cols], cx_a[:, q_sl],
                                         wo_a[:, fsl], start=True, stop=False)
                        nc.tensor.matmul(po[:, 0:cols], cx_b[:, q_sl],
                                         wo_b[:, fsl], start=False, stop=True)
                        nc.vector.tensor_copy(out_t[:, fsl], po[:, 0:cols])
                    nc.sync.dma_start(out=out_o[q_sl, :], in_=out_t)

        persist_cm.__exit__(None, None, None)
        consts_cm.__exit__(None, None, None)

    nc.compile()
    return nc


def _classify_mask(mask: np.ndarray) -> int:
    m = np.asarray(mask)
    if (m != 0).all():
        return MODE_NONE
    tril = np.tril(np.ones((S, S), np.bool_))
    if ((m != 0) == tril).all():
        return MODE_CAUSAL
    return MODE_GENERAL


def _bf(a):
    return np.ascontiguousarray(np.asarray(a, np.float32).astype(
        ml_dtypes.bfloat16))


def _run(inputs: dict, trace: bool = False):
    query = np.asarray(inputs["query"], np.float32)
    key = np.asarray(inputs["key"], np.float32)
    value = np.asarray(inputs["value"], np.float32)
    mask = np.asarray(inputs["mask"])
    wq = np.asarray(inputs["wq"], dtype=np.float32)
    wk = np.asarray(inputs["wk"], dtype=np.float32)
    wv = np.asarray(inputs["wv"], dtype=np.float32)
    wo = np.asarray(inputs["wo"], dtype=np.float32)
    bq = np.asarray(inputs["bq"], dtype=np.float32)
    bk = np.asarray(inputs["bk"], dtype=np.float32)
    bv = np.asarray(inputs["bv"], dtype=np.float32)
    bo = np.asarray(inputs["bo"], dtype=np.float32)

    mode = _classify_mask(mask)
    if mode not in _cache:
        _cache[mode] = _build(mode)
    nc = _cache[mode]

    pen_full = None
    if mode == MODE_GENERAL:
        pen_full = np.where(np.asarray(mask) == 0, np.float32(PEN),
                            np.float32(0.0))

    in_maps = []
    for core in range(N_CORES):
        b = core // HGROUPS
        hg = core % HGROUPS
        r0 = hg * DH
        rs = slice(r0, r0 + DH)
        m = {
            "xq": _bf(query[b]),
            "xk": _bf(key[b]),
            "xv": _bf(value[b]),
            "wqT": _bf(wq[rs, :].T),
            "wkT": _bf(wk[rs, :].T),
            "wvT": _bf(wv[rs, :].T),
            "woT": _bf(wo[:, rs].T),
            "bqd": np.ascontiguousarray(bq[rs].reshape(DH, 1)),
            "bkd": np.ascontiguousarray(bk[rs].reshape(DH, 1)),
            "bvb": np.ascontiguousarray(
                np.broadcast_to(bv[rs][None, :], (P, DH))),
        }
        if mode == MODE_GENERAL:
            m["pen"] = pen_full
        in_maps.append(m)

    res = run_bass_kernel_spmd(nc, in_maps, core_ids=list(range(N_CORES)),
                               trace=trace)

    attn = np.empty((B, H, S, S), np.float32)
    out = np.zeros((B, S, D), np.float32)
    for core in range(N_CORES):
        b = core // HGROUPS
        hg = core % HGROUPS
        r = res.results[core]
        attn[b, hg * HG:(hg + 1) * HG] = r["attn_o"]
        out[b] += r["out_o"]
    out += bo[None, None, :]
    return (out, attn), res


def kernel(**inputs):
    (out, attn), _ = _run(inputs)
    return out, attn


# revision 13
# speedup vs baseline: 2.4965x; 1.0129x over previous
"""Multi-head attention (B=2, S=2048, D=768, H=12) on 8 trn2 NeuronCores.

Sharding: data-parallel over batch (2) x tensor-parallel over heads (4 groups
of 3 heads) = 8 cores. Each core projects Q/K/V for its head group from the
full activations, runs masked softmax attention, writes its slice of the attn
output, and computes a partial output projection (its heads' columns of Wo).
The host sums the 4 partial projections per batch element (the "all-reduce")
and adds the output bias.

Matmul operands are bf16 (fp32 matmuls run as two PE passes on trn2); PSUM
accumulation and the whole softmax/attn-output path stay fp32. Activations are
loaded pre-transposed via xbar DMA transpose (bf16-only HW path).

The mask is classified on the host: causal (tril) -> compile-time triangular
loop bounds, upper triangle of attn left to the runtime's zero-initialized
output buffers; all-ones -> full attention, no penalty; anything else -> an
additive -8e9 penalty tensor is shipped and added to the raw scores.
"""

import contextlib
import math

import ml_dtypes
import numpy as np

import concourse.bass as bass
import concourse.tile as tile
from concourse import bacc, mybir
from concourse.bass_utils import run_bass_kernel_spmd

B = 2
S = 2048
D = 768
H = 12
DK = 64
N_CORES = 8
HGROUPS = N_CORES // B          # 4 head groups
HG = H // HGROUPS               # 3 heads per core
DH = HG * DK                    # 192 projected features per core
SCALE = 1.0 / math.sqrt(DK)
PEN = -8.0e9                    # additive penalty; * SCALE = -1e9 like the ref

F32 = mybir.dt.float32
BF16 = mybir.dt.bfloat16
P = 128                         # partitions
NQT = S // P                    # 16 q tiles
NFC = D // P                    # 6 feature chunks
NSC = S // 512                  # 4 score chunks of 512

MODE_CAUSAL = 0
MODE_NONE = 1
MODE_GENERAL = 2

_cache: dict[int, object] = {}


def _build(mode: int):
    nc = bacc.Bacc("TRN2", target_bir_lowering=False, debug=False,
                   num_devices=N_CORES)

    xq = nc.dram_tensor("xq", [S, D], BF16, kind="ExternalInput")
    xk = nc.dram_tensor("xk", [S, D], BF16, kind="ExternalInput")
    xv = nc.dram_tensor("xv", [S, D], BF16, kind="ExternalInput")
    wqT = nc.dram_tensor("wqT", [D, DH], BF16, kind="ExternalInput")
    wkT = nc.dram_tensor("wkT", [D, DH], BF16, kind="ExternalInput")
    wvT = nc.dram_tensor("wvT", [D, DH], BF16, kind="ExternalInput")
    woT = nc.dram_tensor("woT", [DH, D], BF16, kind="ExternalInput")
    bqd = nc.dram_tensor("bqd", [DH, 1], F32, kind="ExternalInput")
    bkd = nc.dram_tensor("bkd", [DH, 1], F32, kind="ExternalInput")
    bvb = nc.dram_tensor("bvb", [P, DH], F32, kind="ExternalInput")
    pend = None
    if mode == MODE_GENERAL:
        pend = nc.dram_tensor("pen", [S, S], F32, kind="ExternalInput")

    attn_o = nc.dram_tensor("attn_o", [HG, S, S], F32, kind="ExternalOutput")
    out_o = nc.dram_tensor("out_o", [S, D], F32, kind="ExternalOutput")

    ident_d = nc.inline_tensor(
        np.eye(P, dtype=np.float32).astype(ml_dtypes.bfloat16), name="ident")
    # diag-block penalty: 0 where col<=row else PEN (strict upper triangle)
    pen_np = np.where(np.tril(np.ones((P, P), np.bool_)), 0.0, PEN)
    pen_d = nc.inline_tensor(pen_np.astype(np.float32), name="pen_diag")

    Exp = mybir.ActivationFunctionType.Exp
    X = mybir.AxisListType.X

    with tile.TileContext(nc) as tc:
        consts_cm = tc.tile_pool(name="consts", bufs=1)
        consts = consts_cm.__enter__()
        ident = consts.tile([P, P], BF16, tag="ident")
        nc.sync.dma_start(out=ident, in_=ident_d[:, :])
        pen_sb = consts.tile([P, P], F32, tag="pen")
        nc.sync.dma_start(out=pen_sb, in_=pen_d[:, :])
        bq_a = consts.tile([P, 1], F32, tag="bq_a")
        bq_b = consts.tile([DK, 1], F32, tag="bq_b")
        bk_a = consts.tile([P, 1], F32, tag="bk_a")
        bk_b = consts.tile([DK, 1], F32, tag="bk_b")
        nc.sync.dma_start(out=bq_a, in_=bqd[0:P, :])
        nc.sync.dma_start(out=bq_b, in_=bqd[P:DH, :])
        nc.sync.dma_start(out=bk_a, in_=bkd[0:P, :])
        nc.sync.dma_start(out=bk_b, in_=bkd[P:DH, :])
        ones_sb = consts.tile([1, DK], BF16, tag="ones")
        nc.vector.memset(ones_sb, 1.0)
        bv_sb = consts.tile([P, DH], F32, tag="bv")
        nc.sync.dma_start(out=bv_sb, in_=bvb[:, :])

        # weights, [D, DH] viewed as [NFC, 128, DH]
        wq_sb = consts.tile([P, NFC, DH], BF16, tag="wq")
        wk_sb = consts.tile([P, NFC, DH], BF16, tag="wk")
        wv_sb = consts.tile([P, NFC, DH], BF16, tag="wv")
        for (wd, wt) in ((wqT, wq_sb), (wkT, wk_sb), (wvT, wv_sb)):
            wr = wd.rearrange("(c p) m -> c p m", p=P)
            for c in range(NFC):
                nc.sync.dma_start(out=wt[:, c, :], in_=wr[c])
        # woT [DH, D] -> [128, D] + [64, D]
        wo_a = consts.tile([P, D], BF16, tag="wo_a")
        wo_b = consts.tile([DK, D], BF16, tag="wo_b")
        nc.sync.dma_start(out=wo_a, in_=woT[0:P, :])
        nc.sync.dma_start(out=wo_b, in_=woT[P:DH, :])

        # persistent activations
        persist_cm = tc.tile_pool(name="persist", bufs=1)
        persist = persist_cm.__enter__()
        qt_a = persist.tile([P, S], BF16, tag="qt_a")   # heads 0,1 (dk rows)
        qt_b = persist.tile([DK, S], BF16, tag="qt_b")  # head 2
        kt_a = persist.tile([P, S], BF16, tag="kt_a")
        kt_b = persist.tile([DK, S], BF16, tag="kt_b")
        v_sb = persist.tile([P, NQT, DH], BF16, tag="v")  # [s%128, s//128, dh]
        cx_a = persist.tile([P, S], BF16, tag="cx_a")   # ctx^T heads 0,1
        cx_b = persist.tile([DK, S], BF16, tag="cx_b")  # ctx^T head 2

        # ---------------- phase A: projections -----------------------------
        with (
            tc.tile_pool(name="xt", bufs=1) as xtp,
            tc.tile_pool(name="ps_mm", bufs=2, space="PSUM") as ps_mmp,
            tc.tile_pool(name="ps_sm", bufs=2, space="PSUM") as ps_smp,
        ):
            for which, xd in (("k", xk), ("q", xq), ("v", xv)):
                # xbar DMA transpose: x[s, f-chunk] -> xT chunk [128f, S]
                xt_sb = xtp.tile([P, NFC, S], BF16, tag="xt")
                for c in range(NFC):
                    nc.sync.dma_start(out=xt_sb[:, c, :],
                                      in_=xd[:, c * P:(c + 1) * P],
                                      transpose=True)
                if which in ("q", "k"):
                    w_sb = wq_sb if which == "q" else wk_sb
                    b_a = bq_a if which == "q" else bk_a
                    b_b = bq_b if which == "q" else bk_b
                    o_a = qt_a if which == "q" else kt_a
                    o_b = qt_b if which == "q" else kt_b
                    for sc in range(NSC):
                        ssl = slice(sc * 512, (sc + 1) * 512)
                        pa = ps_mmp.tile([P, 512], F32, tag="pm")
                        for c in range(NFC):
                            nc.tensor.matmul(pa, w_sb[:, c, 0:P],
                                             xt_sb[:, c, ssl],
                                             start=(c == 0), stop=(c == NFC - 1))
                        nc.scalar.add(o_a[:, ssl], pa, b_a)
                        pb = ps_smp.tile([P, 512], F32, tag="pb")
                        for c in range(NFC):
                            nc.tensor.matmul(pb[0:DK, :], w_sb[:, c, P:DH],
                                             xt_sb[:, c, ssl],
                                             start=(c == 0), stop=(c == NFC - 1))
                        nc.scalar.add(o_b[:, ssl], pb[0:DK, :], b_b)
                else:
                    for st in range(NQT):
                        pv = ps_smp.tile([P, 512], F32, tag="pb")
                        for c in range(NFC):
                            nc.tensor.matmul(
                                pv[:, 0:DH],
                                xt_sb[:, c, st * P:(st + 1) * P],
                                wv_sb[:, c, :],
                                start=(c == 0), stop=(c == NFC - 1))
                        nc.vector.tensor_add(v_sb[:, st, :], pv[:, 0:DH], bv_sb)

        # ---------------- phase B: attention + output projection ----------
        XBAR_T = False                  # attn^T via DMA xbar (else PE matmul)
        CHW = 512                       # exp/psum chunk width
        NQB = NQT // 4                  # q blocks of 4 q tiles
        with (
            tc.tile_pool(name="exp", bufs=5) as expp,
            tc.tile_pool(name="nrm", bufs=3) as nrmp,
            tc.tile_pool(name="at", bufs=6) as atp,
            tc.tile_pool(name="small", bufs=6) as smallp,
            tc.tile_pool(name="outp", bufs=2) as outp,
            tc.tile_pool(name="penp", bufs=2) as penp,
            tc.tile_pool(name="ps_s", bufs=3, space="PSUM") as ps_sp,
            tc.tile_pool(name="ps_c", bufs=2, space="PSUM") as ps_cp,
            tc.tile_pool(name="ps_o", bufs=1, space="PSUM") as ps_op,
            tc.tile_pool(name="ps_t2", bufs=2, space="PSUM") if not XBAR_T
            else contextlib.nullcontext() as ps_tp2,
        ):
            def q_head(h, sl):
                if h < 2:
                    return qt_a[h * DK:(h + 1) * DK, sl]
                return qt_b[:, sl]

            def k_head(h, sl):
                if h < 2:
                    return kt_a[h * DK:(h + 1) * DK, sl]
                return kt_b[:, sl]

            for qb in range(NQB):
                exps = {}
                recs = {}
                for qi in range(4):
                    qt = qb * 4 + qi
                    q_sl = slice(qt * P, (qt + 1) * P)
                    ncol = (qt + 1) * P if mode == MODE_CAUSAL else S
                    nch = (ncol + CHW - 1) // CHW

                    pg_t = None
                    if mode == MODE_GENERAL:
                        pg_t = penp.tile([P, S], F32, tag="pg")
                        nc.sync.dma_start(out=pg_t, in_=pend[q_sl, :])

                    exp_h = [expp.tile([P, S], BF16, tag=f"exp{h}",
                                       name=f"exp{h}") for h in range(HG)]
                    acc_h = [smallp.tile([P, S // CHW], F32, tag=f"acc{h}",
                                        name=f"acc{h}") for h in range(HG)]
                    for h in range(HG):
                        exps[(h, qi)] = exp_h[h]
                    for ch in range(nch):
                        ccols = min(CHW, ncol - ch * CHW)
                        nsub = (ccols + 511) // 512
                        ps_h = [ps_sp.tile([P, CHW], F32, tag="ps",
                                          name=f"ps{h2_}")
                                for h2_ in range(HG)]
                        # h0/h1 adjacent -> concurrent PE row groups
                        for sub in range(nsub):
                            cols = min(512, ccols - sub * 512)
                            c0 = ch * CHW + sub * 512
                            for h in range(HG):
                                nc.tensor.matmul(
                                    ps_h[h][:, sub * 512:sub * 512 + cols],
                                    q_head(h, q_sl), k_head(h, slice(c0, c0 + cols)),
                                    start=True, stop=True)
                        for h in range(HG):
                            if mode == MODE_CAUSAL and (qt * P) // CHW == ch:
                                off = qt * P - ch * CHW
                                nc.vector.tensor_add(ps_h[h][:, off:off + P],
                                                     ps_h[h][:, off:off + P],
                                                     pen_sb)
                            elif mode == MODE_GENERAL:
                                nc.vector.tensor_add(
                                    ps_h[h][:, 0:ccols], ps_h[h][:, 0:ccols],
                                    pg_t[:, ch * CHW:ch * CHW + ccols])
                            nc.scalar.activation(
                                exp_h[h][:, ch * CHW:ch * CHW + ccols],
                                ps_h[h][:, 0:ccols], Exp, scale=SCALE,
                                accum_out=acc_h[h][:, ch:ch + 1])

                    for h in range(HG):
                        red_t = smallp.tile([P, 1], F32, tag="red")
                        if nch > 1:
                            nc.vector.reduce_sum(red_t, acc_h[h][:, 0:nch],
                                                 axis=X)
                        else:
                            nc.vector.tensor_copy(red_t, acc_h[h][:, 0:1])
                        rec_t = smallp.tile([P, 1], F32, tag="rec")
                        nc.vector.reciprocal(rec_t, red_t)
                        rec_bf = smallp.tile([P, 1], BF16, tag=f"rb{h}")
                        nc.vector.tensor_copy(rec_bf, rec_t)
                        recs[(h, qi)] = rec_bf

                        nrm_t = nrmp.tile([P, S], F32, tag="nrm")
                        nc.vector.tensor_scalar_mul(nrm_t[:, 0:ncol],
                                                    exp_h[h][:, 0:ncol], rec_t)
                        nc.sync.dma_start(out=attn_o[h, q_sl, 0:ncol],
                                          in_=nrm_t[:, 0:ncol])

                # attn^T + ctx + recip scaling + cx, per head
                nkcb = qb * 4 + 4 if mode == MODE_CAUSAL else NQT
                qb_sl = slice(qb * 512, (qb + 1) * 512)
                for h in range(HG):
                    pc4 = ps_cp.tile([DK, 512], F32, tag="pc")
                    for kc in range(nkcb):
                        qi0 = max(kc - qb * 4, 0) if mode == MODE_CAUSAL else 0
                        at4 = atp.tile([P, 512], BF16, tag="at4")
                        if XBAR_T:
                            for qi in range(qi0, 4):
                                nc.sync.dma_start(
                                    out=at4[:, qi * P:(qi + 1) * P],
                                    in_=exps[(h, qi)][:, kc * P:(kc + 1) * P],
                                    transpose=True)
                        else:
                            pt2 = ps_tp2.tile([P, 512], F32, tag="pt2")
                            for qi in range(qi0, 4):
                                nc.tensor.matmul(
                                    pt2[:, qi * P:(qi + 1) * P],
                                    exps[(h, qi)][:, kc * P:(kc + 1) * P],
                                    ident, start=True, stop=True)
                            if kc % 2 == 0:
                                nc.scalar.copy(at4[:, qi0 * P:512],
                                               pt2[:, qi0 * P:512])
                            else:
                                nc.vector.tensor_copy(at4[:, qi0 * P:512],
                                                      pt2[:, qi0 * P:512])
                        nc.tensor.matmul(
                            pc4[:, qi0 * P:512],
                            v_sb[:, kc, h * DK:(h + 1) * DK],
                            at4[:, qi0 * P:512],
                            start=(kc == 0), stop=(kc == nkcb - 1),
                            skip_group_check=True)

                    # rb4 = ones(64)^T x recip-row(512), applied to pc4
                    recT_ps = ps_op.tile([1, 512], F32, tag="po")
                    for qi in range(4):
                        nc.tensor.matmul(recT_ps[:, qi * P:(qi + 1) * P],
                                         recs[(h, qi)], ident,
                                         start=True, stop=True)
                    recT_sb = smallp.tile([1, 512], BF16, tag="rt")
                    nc.scalar.copy(recT_sb, recT_ps)
                    rb_ps = ps_op.tile([DK, 512], F32, tag="po")
                    nc.tensor.matmul(rb_ps, ones_sb, recT_sb,
                                     start=True, stop=True)
                    rb_sb = smallp.tile([DK, 512], F32, tag="rbs")
                    nc.scalar.copy(rb_sb, rb_ps)
                    if h < 2:
                        nc.vector.tensor_mul(cx_a[h * DK:(h + 1) * DK, qb_sl],
                                             pc4, rb_sb)
                    else:
                        nc.vector.tensor_mul(cx_b[:, qb_sl], pc4, rb_sb)

                # output projection for the 4 q tiles of this block
                for qi in range(4):
                    qt = qb * 4 + qi
                    q_sl = slice(qt * P, (qt + 1) * P)
                    out_t = outp.tile([P, D], F32, tag="out")
                    for fo in range(2):
                        cols = 512 if fo == 0 else D - 512
                        fsl = slice(fo * 512, fo * 512 + cols)
                        po = ps_op.tile([P, 512], F32, tag="po")
                        nc.tensor.matmul(po[:, 0:cols], cx_a[:, q_sl],
                                         wo_a[:, fsl], start=True, stop=False)
                        nc.tensor.matmul(po[:, 0:cols], cx_b[:, q_sl],
                                         wo_b[:, fsl], start=False, stop=True)
                        nc.vector.tensor_copy(out_t[:, fsl], po[:, 0:cols])
                    nc.sync.dma_start(out=out_o[q_sl, :], in_=out_t)

        persist_cm.__exit__(None, None, None)
        consts_cm.__exit__(None, None, None)

    nc.compile()
    return nc


def _classify_mask(mask: np.ndarray) -> int:
    m = np.asarray(mask)
    if (m != 0).all():
        return MODE_NONE
    tril = np.tril(np.ones((S, S), np.bool_))
    if ((m != 0) == tril).all():
        return MODE_CAUSAL
    return MODE_GENERAL


def _bf(a):
    return np.ascontiguousarray(np.asarray(a, np.float32).astype(
        ml_dtypes.bfloat16))


def _run(inputs: dict, trace: bool = False):
    query = np.asarray(inputs["query"], np.float32)
    key = np.asarray(inputs["key"], np.float32)
    value = np.asarray(inputs["value"], np.float32)
    mask = np.asarray(inputs["mask"])
    wq = np.asarray(inputs["wq"], dtype=np.float32)
    wk = np.asarray(inputs["wk"], dtype=np.float32)
    wv = np.asarray(inputs["wv"], dtype=np.float32)
    wo = np.asarray(inputs["wo"], dtype=np.float32)
    bq = np.asarray(inputs["bq"], dtype=np.float32)
    bk = np.asarray(inputs["bk"], dtype=np.float32)
    bv = np.asarray(inputs["bv"], dtype=np.float32)
    bo = np.asarray(inputs["bo"], dtype=np.float32)

    mode = _classify_mask(mask)
    if mode not in _cache:
        _cache[mode] = _build(mode)
    nc = _cache[mode]

    pen_full = None
    if mode == MODE_GENERAL:
        pen_full = np.where(np.asarray(mask) == 0, np.float32(PEN),
                            np.float32(0.0))

    in_maps = []
    for core in range(N_CORES):
        b = core // HGROUPS
        hg = core % HGROUPS
        r0 = hg * DH
        rs = slice(r0, r0 + DH)
        m = {
            "xq": _bf(query[b]),
            "xk": _bf(key[b]),
            "xv": _bf(value[b]),
            "wqT": _bf(wq[rs, :].T),
            "wkT": _bf(wk[rs, :].T),
            "wvT": _bf(wv[rs, :].T),
            "woT": _bf(wo[:, rs].T),
            "bqd": np.ascontiguousarray(bq[rs].reshape(DH, 1)),
            "bkd": np.ascontiguousarray(bk[rs].reshape(DH, 1)),
            "bvb": np.ascontiguousarray(
                np.broadcast_to(bv[rs][None, :], (P, DH))),
        }
        if mode == MODE_GENERAL:
            m["pen"] = pen_full
        in_maps.append(m)

    res = run_bass_kernel_spmd(nc, in_maps, core_ids=list(range(N_CORES)),
                               trace=trace)

    attn = np.empty((B, H, S, S), np.float32)
    out = np.zeros((B, S, D), np.float32)
    for core in range(N_CORES):
        b = core // HGROUPS
        hg = core % HGROUPS
        r = res.results[core]
        attn[b, hg * HG:(hg + 1) * HG] = r["attn_o"]
        out[b] += r["out_o"]
    out += bo[None, None, :]
    return (out, attn), res


def kernel(**inputs):
    (out, attn), _ = _run(inputs)
    return out, attn
